# revision 26
# baseline (speedup 1.0000x reference)
"""Multi-head linear attention (elu+1 feature map) on 8 TRN2 NeuronCores.

Sharding: core c handles batch b = c//2, sequence half j = c%2 (2048 rows).
Each core computes q/k/v projections + phi + partial kv/z for its rows,
AllReduces kv/z across the (b, j) pair, then computes num/den/ctx and the
output projection for its rows. All matmuls in bf16 (fp32 PSUM accumulate).

Device-side layout notes:
  - query slice arrives host-transposed (feature-major) in pair-blocks
    (128 = 2 heads x 64 dims, S) so projections contract the feature dim
    on partitions with block-diagonal weights.
  - K1: one N=256 matmul per s-chunk projects k|v together and one N=129
    matmul accumulates kv|z together (rhs = [v | ones]), halving the
    LDWEIGHTS stream versus separate matmuls.
  - phi(x) = elu(x)+1 = min(exp(x),1) + relu(x): Exp and Relu LUT passes
    on Scalar, fused (min,add) scalar_tensor_tensor merge on Vector.
  - The AllReduce payload carries only the two diagonal 64x64 kv blocks
    plus the z column per pair ([128, 65] bf16 per pair).
  - qf (feature-major) is computed while the AllReduce is in flight;
    warm-keep matmuls paced by qf keep the PE HAM un-throttled.
  - den rows for all 16 heads accumulate in one PSUM tile via zero-padded
    z columns; den/recip processing is split by sequence half so the
    reciprocal broadcast DMA overlaps the second half's accumulation.
  - num/O interleave by sequence half; division by den is fused into the
    num PSUM eviction against a partition-broadcast reciprocal.
"""
import numpy as np
import ml_dtypes

B, S, H, Dh = 4, 4096, 16, 64
E = H * Dh
N_CORES = 8
SL = S // 2          # sequence rows per core
NPAIR = H // 2       # head pairs
EPS = 1e-6

_CACHE = {}


def _build_program():
    import concourse.bacc as bacc
    import concourse.mybir as mybir
    import concourse.tile as tile

    bf16 = mybir.dt.bfloat16
    f32 = mybir.dt.float32
    Act = mybir.ActivationFunctionType
    Alu = mybir.AluOpType

    nc = bacc.Bacc(None, target_bir_lowering=False, num_devices=N_CORES)

    xq = nc.dram_tensor("xqT", [E, SL], bf16, kind="ExternalInput")
    wq_bd = nc.dram_tensor("wq_bd", [NPAIR, 128, 128], bf16, kind="ExternalInput")
    wkv_bd = nc.dram_tensor("wkv_bd", [NPAIR, 128, 256], bf16, kind="ExternalInput")
    wo = nc.dram_tensor("wo", [E, E], bf16, kind="ExternalInput")
    y = nc.dram_tensor("y", [SL, E], f32, kind="ExternalOutput")
    kv_ar = nc.dram_tensor("kv_ar", [128, (NPAIR // 2) * 65], bf16)
    kv_ar2 = nc.dram_tensor("kv_ar2", [128, (NPAIR // 2) * 65], bf16)

    NCHUNK = SL // 128   # s-chunks per pair (16)
    NQC = SL // 512
    GC = 4               # chunks per phi/eviction group

    with tile.TileContext(nc) as tc:
        with (
            tc.tile_pool(name="persist", bufs=1) as persist,
            tc.tile_pool(name="xp", bufs=1) as xp,
            tc.tile_pool(name="kvsb", bufs=2) as kvsb,
            tc.tile_pool(name="tmp", bufs=3) as tmp,
            tc.tile_pool(name="rbcp", bufs=4) as rbcp,
            tc.tile_pool(name="outp", bufs=2) as outp,
            tc.tile_pool(name="dram", bufs=1, space="DRAM") as dram,
        ):
            # ---- weights / constants ----
            wkv_sb = persist.tile([128, NPAIR, 256], bf16)
            nc.sync.dma_start(out=wkv_sb[:], in_=wkv_bd.rearrange("p k m -> k p m"))
            xTs = []
            for p in range(NPAIR):
                xT = xp.tile([128, SL], bf16, tag=f"xT{p}")
                nc.sync.dma_start(out=xT[:], in_=xq[p * 128:(p + 1) * 128, :])
                xTs.append(xT)
            wq_sb = persist.tile([128, NPAIR, 128], bf16)
            nc.gpsimd.dma_start(out=wq_sb[:], in_=wq_bd.rearrange("p k m -> k p m"))
            wo_sb = persist.tile([128, NPAIR, E], bf16)
            nc.gpsimd.dma_start(
                out=wo_sb[:], in_=wo.rearrange("(k p) n -> p k n", p=128)
            )
            qfT = persist.tile([128, NPAIR, SL], bf16)
            ctxT = persist.tile([128, NPAIR, SL], bf16)
            eps_sb = persist.tile([16, 1], f32)
            nc.vector.memset(eps_sb[:], EPS)

            # ---- phase K1: kf/v (s-major) + kv/z for every pair ----
            HKV = (NPAIR // 2) * 65
            kv_in = dram.tile([128, HKV], bf16)
            kv_in2 = dram.tile([128, HKV], bf16)
            groups = [[0, 1], [2, 3], [4, 5], [6, 7]]
            with (
                tc.tile_pool(name="ps_kvp", bufs=3, space="PSUM") as ps_kvp,
                tc.tile_pool(name="ps_kv", bufs=2, space="PSUM") as ps_kv,
            ):
                NG = NCHUNK // GC  # 4 groups per pair
                for p in range(NPAIR):
                    xT = xTs[p]
                    kf = kvsb.tile([128, NCHUNK, 128], bf16, tag="kf")
                    vz = kvsb.tile([128, NCHUNK, 129], bf16, tag="vz")
                    nc.vector.memset(vz[:, :, 128:129], 1.0)
                    kvzacc = ps_kv.tile([128, 129], f32, tag="kvz")

                    def proj_group(g):
                        kvps = ps_kvp.tile([128, GC, 256], f32, tag="kvps")
                        for c in range(GC):
                            i = g * GC + c
                            nc.tensor.matmul(
                                kvps[:, c, :],
                                lhsT=xT[:, i * 128:(i + 1) * 128],
                                rhs=wkv_sb[:, p, :],
                                start=True, stop=True,
                            )
                        return kvps

                    def phi_group(g, kvps):
                        cs = slice(g * GC, (g + 1) * GC)
                        E1 = tmp.tile([128, GC, 128], bf16, tag="E1")
                        nc.scalar.activation(E1[:], kvps[:, :, 0:128], Act.Exp)
                        # Em = min(exp(x), 1) reads SBUF bf16 at the 2x rate;
                        # the merge reads x once more from PSUM (1x anyway).
                        Em = tmp.tile([128, GC, 128], bf16, tag="R1")
                        nc.vector.tensor_scalar_min(Em[:], E1[:], 1.0)
                        nc.vector.scalar_tensor_tensor(
                            kf[:, cs, :], kvps[:, :, 0:128], 1.0, Em[:],
                            Alu.add, Alu.max,
                        )
                        nc.scalar.copy(vz[:, cs, 0:128], kvps[:, :, 128:256])

                    def acc_group(g):
                        for c in range(GC):
                            i = g * GC + c
                            nc.tensor.matmul(
                                kvzacc[:],
                                lhsT=kf[:, i, :], rhs=vz[:, i, :],
                                start=(i == 0), stop=(i == NCHUNK - 1),
                            )

                    # software pipeline: P0 P1 A0 P2 A1 P3 A2 A3
                    ps = [proj_group(0)]
                    phi_group(0, ps[0])
                    ps.append(proj_group(1))
                    phi_group(1, ps[1])
                    acc_group(0)
                    ps.append(proj_group(2))
                    phi_group(2, ps[2])
                    acc_group(1)
                    ps.append(proj_group(3))
                    phi_group(3, ps[3])
                    acc_group(2)
                    acc_group(3)

                    kvst = outp.tile([128, 65], bf16, tag="kvst")
                    nc.vector.tensor_copy(kvst[0:64, 0:64], kvzacc[0:64, 0:64])
                    nc.vector.tensor_copy(
                        kvst[64:128, 0:64], kvzacc[64:128, 64:128]
                    )
                    nc.vector.tensor_copy(kvst[:, 64:65], kvzacc[:, 128:129])
                    tgt = kv_in if p < NPAIR // 2 else kv_in2
                    nc.sync.dma_start(
                        out=tgt[:, (p % (NPAIR // 2)) * 65:
                                (p % (NPAIR // 2) + 1) * 65],
                        in_=kvst[:],
                    )
                    if p == NPAIR // 2 - 1:
                        # first-half AllReduce fires mid-K1; its ~25us
                        # latency hides entirely under the second half.
                        nc.gpsimd.collective_compute(
                            "AllReduce", Alu.add, replica_groups=groups,
                            ins=[kv_in[:]], outs=[kv_ar[:]],
                        )

            # ---- phase R: AllReduce kv/z second half ----
            nc.gpsimd.collective_compute(
                "AllReduce", Alu.add, replica_groups=groups,
                ins=[kv_in2[:]], outs=[kv_ar2[:]],
            )
            kvrd = persist.tile([128, NPAIR, 65], bf16)
            kvbd = persist.tile([128, NPAIR, 128], bf16)
            nc.vector.memset(kvbd[:], 0.0)
            zbd = persist.tile([128, NPAIR, H], bf16)
            nc.vector.memset(zbd[:], 0.0)
            nc.scalar.dma_start(
                out=kvrd[:, 0:NPAIR // 2, :],
                in_=kv_ar.rearrange("q (p c) -> q p c", c=65),
            )
            # half-1 kvbd/zbd staging rides the idle SP DMA queue while the
            # compute engines are still busy with qf; this lets den start
            # for pairs 0-3 right after the first collective lands.
            nc.sync.dma_start(
                out=kvbd[0:64, 0:4, 0:64], in_=kvrd[0:64, 0:4, 0:64]
            )
            nc.sync.dma_start(
                out=kvbd[64:128, 0:4, 64:128], in_=kvrd[64:128, 0:4, 0:64]
            )
            for p in range(4):
                nc.sync.dma_start(
                    out=zbd[0:64, p, 2 * p:2 * p + 1], in_=kvrd[0:64, p, 64:65]
                )
                nc.sync.dma_start(
                    out=zbd[64:128, p, 2 * p + 1:2 * p + 2],
                    in_=kvrd[64:128, p, 64:65],
                )
            nc.sync.dma_start(
                out=kvrd[:, NPAIR // 2:, :],
                in_=kv_ar2.rearrange("q (p c) -> q p c", c=65),
            )

            # ---- phase K2: qf (feature-major), overlapping the collective --
            with (
                tc.tile_pool(name="ps_q", bufs=3, space="PSUM") as ps_q,
                tc.tile_pool(name="ps_warm", bufs=1, space="PSUM") as ps_warm,
            ):
                for p in range(NPAIR):
                    xT = xTs[p]
                    for qc in range(2):
                        qs = slice(qc * 1024, (qc + 1) * 1024)
                        qps = ps_q.tile([128, 1024], f32, tag="qps")
                        nc.tensor.matmul(
                            qps[:, 0:512], lhsT=wq_sb[:, p, :],
                            rhs=xT[:, qc * 1024:qc * 1024 + 512],
                            start=True, stop=True,
                        )
                        nc.tensor.matmul(
                            qps[:, 512:1024], lhsT=wq_sb[:, p, :],
                            rhs=xT[:, qc * 1024 + 512:(qc + 1) * 1024],
                            start=True, stop=True,
                        )
                        qE = tmp.tile([128, 1024], bf16, tag="E1")
                        nc.scalar.activation(qE[:], qps[:], Act.Exp)
                        qM = tmp.tile([128, 1024], bf16, tag="R1")
                        if p % 4 == 3 and qc == 1:
                            nc.scalar.activation(qM[:], qps[:], Act.Relu)
                            nc.vector.scalar_tensor_tensor(
                                qfT[:, p, qs], qE[:], 1.0, qM[:],
                                Alu.min, Alu.add,
                            )
                        else:
                            nc.vector.tensor_scalar_min(qM[:], qE[:], 1.0)
                            nc.vector.scalar_tensor_tensor(
                                qfT[:, p, qs], qps[:], 1.0, qM[:],
                                Alu.add, Alu.max,
                            )
                # warm-keep: paced by qf completion so the PE HAM stays hot
                # through the collective wait.
                warm = ps_warm.tile([128, 512], f32)
                for p in range(NPAIR - 2):
                    nc.tensor.matmul(
                        warm[:], lhsT=qfT[:, p, 0:128], rhs=qfT[:, p, 0:512],
                        start=True, stop=True,
                    )

            # small post-collective copies on Scalar (Vector feeds the ctx
            # pipeline soon after).  tile_wait_until pins the whole tail into
            # strict scheduler stages: the static cost model underestimates
            # the collective latency, and without the pins it interleaves
            # ctx-gated num/O matmuls into the den chain's semaphore window,
            # convoying the PE queue on real hardware.
            tc.tile_set_cur_wait(0.30)
            pslc = slice(NPAIR // 2, NPAIR)
            nc.scalar.copy(kvbd[0:64, pslc, 0:64], kvrd[0:64, pslc, 0:64])
            nc.scalar.copy(kvbd[64:128, pslc, 64:128], kvrd[64:128, pslc, 0:64])
            for p in range(NPAIR // 2, NPAIR):
                if p % 2 == 0:
                    nc.vector.tensor_copy(
                        zbd[0:64, p, 2 * p:2 * p + 1], kvrd[0:64, p, 64:65]
                    )
                    nc.vector.tensor_copy(
                        zbd[64:128, p, 2 * p + 1:2 * p + 2], kvrd[64:128, p, 64:65]
                    )
                else:
                    nc.scalar.copy(
                        zbd[0:64, p, 2 * p:2 * p + 1], kvrd[0:64, p, 64:65]
                    )
                    nc.scalar.copy(
                        zbd[64:128, p, 2 * p + 1:2 * p + 2], kvrd[64:128, p, 64:65]
                    )

            # ---- den accumulation, processed per sequence half so the
            # reciprocal broadcast overlaps the second half ----
            recip_dram = dram.tile([16, SL], bf16)
            den_sb = persist.tile([16, SL], bf16)
            recip_bf = persist.tile([16, SL], bf16)
            with tc.tile_pool(name="ps_den", bufs=1, space="PSUM") as ps_den:
                denps = ps_den.tile([16, SL], f32)
                for qc in range(2):
                    tc.tile_set_cur_wait(0.31 + 0.01 * qc)
                    qs2 = slice(qc * 1024, (qc + 1) * 1024)
                    for p in range(NPAIR):
                        for q5 in range(2):
                            qs = slice(qc * 1024 + q5 * 512,
                                       qc * 1024 + (q5 + 1) * 512)
                            nc.tensor.matmul(
                                denps[:, qs], lhsT=zbd[:, p, :],
                                rhs=qfT[:, p, qs],
                                start=(p == 0), stop=(p == NPAIR - 1),
                            )
                    nc.scalar.activation(
                        den_sb[:, qs2], denps[:, qs2], Act.Identity, bias=eps_sb[:]
                    )
                    eng = nc.scalar
                    eng.add_instruction(
                        mybir.InstActivation(
                            name=nc.get_next_instruction_name(),
                            func=Act.Reciprocal,
                            ins=[
                                eng.lower_ap(den_sb[:, qs2]),
                                mybir.ImmediateValue(dtype=f32, value=0.0),
                                mybir.ImmediateValue(dtype=f32, value=1.0),
                                mybir.ImmediateValue(dtype=f32, value=0.0),
                            ],
                            outs=[eng.lower_ap(recip_bf[:, qs2])],
                        )
                    )
                    nc.sync.dma_start(
                        out=recip_dram[:, qs2], in_=recip_bf[:, qs2]
                    )


            # ---- phases N2/O interleaved by sequence half ----
            with (
                tc.tile_pool(name="ps_num", bufs=4, space="PSUM") as ps_num,
                tc.tile_pool(name="ps_o", bufs=2, space="PSUM") as ps_o,
            ):
                def emit_num(qc, p):
                    qs = slice(qc * 1024, (qc + 1) * 1024)
                    rbc = rbcp.tile([128, 1024], bf16, tag="rbc")
                    nc.sync.dma_start(
                        out=rbc[0:64, :],
                        in_=recip_dram[2 * p:2 * p + 1, qs].to_broadcast(
                            [64, 1024]
                        ),
                    )
                    nc.scalar.dma_start(
                        out=rbc[64:128, :],
                        in_=recip_dram[2 * p + 1:2 * p + 2, qs].to_broadcast(
                            [64, 1024]
                        ),
                    )
                    # 512-wide PSUM tiles keep four num matmuls in flight,
                    # halving the nps->ctx round-trip pacing.
                    for h in range(2):
                        hs = slice(qc * 1024 + h * 512, qc * 1024 + (h + 1) * 512)
                        nps = ps_num.tile([128, 512], f32, tag="nps")
                        nc.tensor.matmul(
                            nps[:], lhsT=kvbd[:, p, :], rhs=qfT[:, p, hs],
                            start=True, stop=True,
                        )
                        nc.vector.tensor_tensor(
                            ctxT[:, p, hs], nps[:], rbc[:, h * 512:(h + 1) * 512],
                            Alu.mult,
                        )

                def o_chunk(opst, sis, k):
                    for ops, si in zip(opst, sis):
                        ss = slice(si * 128, (si + 1) * 128)
                        nc.tensor.matmul(
                            ops[:, 0:512], lhsT=ctxT[:, k, ss],
                            rhs=wo_sb[:, k, 0:512],
                            start=(k == 0), stop=(k == NPAIR - 1),
                        )
                        nc.tensor.matmul(
                            ops[:, 512:E], lhsT=ctxT[:, k, ss],
                            rhs=wo_sb[:, k, 512:E],
                            start=(k == 0), stop=(k == NPAIR - 1),
                        )

                def o_evict(opst, sis):
                    for ops, si in zip(opst, sis):
                        ss = slice(si * 128, (si + 1) * 128)
                        ysb = outp.tile([128, E], f32, tag="ysb")
                        nc.vector.tensor_copy(ysb[:, 0:512], ops[:, 0:512])
                        nc.scalar.copy(ysb[:, 512:E], ops[:, 512:E])
                        nc.sync.dma_start(out=y[ss, 0:512], in_=ysb[:, 0:512])
                        nc.scalar.dma_start(out=y[ss, 512:E], in_=ysb[:, 512:E])

                def o_pair(qc, sp, interleave=None):
                    sis = (qc * 8 + 2 * sp, qc * 8 + 2 * sp + 1)
                    ops_a = ps_o.tile([128, E], f32, tag="ops")
                    ops_b = ps_o.tile([128, E], f32, tag="ops")
                    opst = [ops_a, ops_b]
                    for k in range(NPAIR):
                        o_chunk(opst, sis, k)
                        if interleave is not None and k + 2 < NPAIR:
                            interleave(k + 2)
                    o_evict(opst, sis)

                # qc0: num stream with the first O si-pair riding along
                tc.tile_set_cur_wait(0.34)
                emit_num(0, 0)
                emit_num(0, 1)
                o_pair(0, 0, interleave=lambda p: emit_num(0, p))
                o_pair(0, 1)
                # qc1 num runs on the PE between qc0's O chunks so its ctx
                # is ready before the qc1 output projection begins
                tc.tile_set_cur_wait(0.345)
                for p in range(NPAIR):
                    emit_num(1, p)
                tc.tile_set_cur_wait(0.346)
                o_pair(0, 2)
                o_pair(0, 3)
                tc.tile_set_cur_wait(0.35)
                for sp in range(4):
                    o_pair(1, sp)

    nc.compile()
    return nc


def _get_program():
    if "nc" not in _CACHE:
        _CACHE["nc"] = _build_program()
    return _CACHE["nc"]


def _host_prep(query, Wq, Wk, Wv, Wo):
    bf16 = ml_dtypes.bfloat16
    q_bf = np.ascontiguousarray(query.astype(bf16))
    wq_bd = np.zeros((NPAIR, 128, 128), dtype=bf16)
    wkv_bd = np.zeros((NPAIR, 128, 256), dtype=bf16)
    for p in range(NPAIR):
        wq_bd[p, 0:64, 0:64] = Wq[2 * p]
        wq_bd[p, 64:128, 64:128] = Wq[2 * p + 1]
        wkv_bd[p, 0:64, 0:64] = Wk[2 * p]
        wkv_bd[p, 64:128, 64:128] = Wk[2 * p + 1]
        wkv_bd[p, 0:64, 128:192] = Wv[2 * p]
        wkv_bd[p, 64:128, 192:256] = Wv[2 * p + 1]
    wo_bf = np.ascontiguousarray(Wo.astype(bf16))
    in_maps = []
    for c in range(N_CORES):
        b, j = divmod(c, 2)
        in_maps.append({
            "xqT": np.ascontiguousarray(q_bf[b, j * SL:(j + 1) * SL, :].T),
            "wq_bd": wq_bd,
            "wkv_bd": wkv_bd,
            "wo": wo_bf,
        })
    return in_maps


def kernel(query, Wq, Wk, Wv, Wo):
    from concourse.bass_utils import run_bass_kernel_spmd

    nc = _get_program()
    in_maps = _host_prep(query, Wq, Wk, Wv, Wo)
    res = run_bass_kernel_spmd(nc, in_maps, list(range(N_CORES)))
    out = np.empty((B, S, E), dtype=np.float32)
    for c in range(N_CORES):
        b, j = divmod(c, 2)
        out[b, j * SL:(j + 1) * SL, :] = res.results[c]["y"]
    return out
